# revision 10
# baseline (speedup 1.0000x reference)
"""Causal single-head attention (B=8, N=2048, D=H=1024, fp32) on 8 TRN2 cores.

Data-parallel: one batch element per NeuronCore. All matmuls run as fp8
(e4m3) DoubleRow matmuls — 256-deep contraction at 0.5 cycles/row, 4x the
fp32r/bf16 MAC rate — with fp32 PSUM accumulation. Accuracy is recovered
with residual ("hi+lo") fp8 splits on the error-critical paths:

  q = (x8 + xr8) @ Wq8 + bq        (2-chain; W pre-scaled by 16 so its
  k = (x8 + xr8) @ Wk8             uniform(-1/32,1/32) values quantize in
  v = (x8+xr8)@Wv8 + x8@Wvr8       e4m3's normal range; 3-chain for V)
  s = (q8 + qr8) . k8              (q eviction split; K bias dropped —
                                   softmax-invariant)
  p = exp(s/(32*256)) -> bf16 -> p8 + pr8
  out = [(p8+pr8).(v8+vr8-ish)] / rowsum + bv   (3-chain AV, bias applied
                                   after normalization — it commutes)

Everything stays resident in SBUF (no DRAM spills): per-core HBM traffic
is ~8.5 MB in (fp8 inputs) + 8 MB out vs ~60 MB for the fp32r version.
"""

import os
import sys
from contextlib import ExitStack

import numpy as np
import ml_dtypes

# The concourse/bass toolchain comes from the container's python path; fall
# back to the /opt copy when running outside the preconfigured interpreter.
try:
    import concourse.bacc as bacc
except ImportError:  # pragma: no cover
    sys.path.insert(0, "/opt/trn_rl_repo")
    import concourse.bacc as bacc

import concourse.mybir as mybir
from concourse.tile import TileContext
from concourse.bass_utils import run_bass_kernel_spmd

# bass_utils imports antenv.axon_hooks when BASS_TRACE is set; provide a stub
# so tracing degrades gracefully instead of crashing if the module is absent.
try:
    import antenv.axon_hooks  # noqa: F401
except ImportError:  # pragma: no cover
    import types

    _m = types.ModuleType("antenv.axon_hooks")
    _m._hook = None
    _m.set_axon_ntff_profile_hook = lambda h: setattr(_m, "_hook", h)
    _m.get_axon_ntff_profile_hook = lambda: _m._hook
    sys.modules["antenv.axon_hooks"] = _m

# The boot-time NTFF hook install degrades silently when the image's antenv
# lacks axon_hooks; re-attempt it against our stub so BASS_TRACE captures
# HW profiles. Harmless no-op when axon or the .so is absent.
try:  # pragma: no cover
    import antenv.axon_hooks as _ah

    if _ah.get_axon_ntff_profile_hook() is None:
        from trn_agent_boot.trn_boot import _ntff_profile_via_ctypes

        _hook = _ntff_profile_via_ctypes("/opt/axon/libaxon_pjrt.so")
        if _hook is not None:
            _ah.set_axon_ntff_profile_hook(_hook)
except Exception:
    pass

B, N, D, H = 8, 2048, 1024, 1024
P = 128
DP = D // (2 * P)    # 4 contraction pair-tiles (256 deep each)
HP = H // (2 * P)    # 4 h pair-tiles for the score contraction
NT = N // P          # 16 sequence tiles of 128
IT = N // 512        # 4 query tiles of 512
WS = 16.0            # weight pre-scale: keeps W out of e4m3 subnormals
EXP_SCALE = 1.0 / (np.sqrt(float(H)) * WS * WS)  # 2^-13

F32 = mybir.dt.float32
F8 = mybir.dt.float8e4
BF16 = mybir.dt.bfloat16
F8NP = ml_dtypes.float8_e4m3
DR = mybir.MatmulPerfMode.DoubleRow

QK_CHAINS = int(os.environ.get("ATTN_QK_CHAINS", "2"))  # 1 or 2
QRES = os.environ.get("ATTN_QRES", "1") == "1"

LAST_RESULT = None  # BassKernelResults of the most recent kernel() call
_CACHE = {}


def build_program(qk_chains: int = QK_CHAINS, qres: bool = QRES, debug_dump: bool = False):
    nc = bacc.Bacc("TRN2", target_bir_lowering=False, debug=False)

    x8d = nc.dram_tensor("x8d", [DP, P, 2, N], F8, kind="ExternalInput")
    xr8d = nc.dram_tensor("xr8d", [DP, P, 2, N], F8, kind="ExternalInput")
    wq8d = nc.dram_tensor("wq8d", [DP, P, 2, H], F8, kind="ExternalInput")
    wk8d = nc.dram_tensor("wk8d", [DP, P, 2, H], F8, kind="ExternalInput")
    wv8d = nc.dram_tensor("wv8d", [DP, P, 2, H], F8, kind="ExternalInput")
    wvr8d = nc.dram_tensor("wvr8d", [DP, P, 2, H], F8, kind="ExternalInput")
    bqS = nc.dram_tensor("bqS", [P, 8], F32, kind="ExternalInput")
    bvB = nc.dram_tensor("bvB", [P, H], F32, kind="ExternalInput")
    out = nc.dram_tensor("out", [N, H], F32, kind="ExternalOutput")
    if debug_dump:
        dbg_q = nc.dram_tensor("dbg_q", [HP, P, 2, N], F8, kind="ExternalOutput")
        dbg_qr = nc.dram_tensor("dbg_qr", [HP, P, 2, N], F8, kind="ExternalOutput")
        dbg_k = nc.dram_tensor("dbg_k", [HP, P, 2, N], F8, kind="ExternalOutput")
        dbg_v = nc.dram_tensor("dbg_v", [NT // 2, P, 2, H], F8, kind="ExternalOutput")
        dbg_vr = nc.dram_tensor("dbg_vr", [NT // 2, P, 2, H], F8, kind="ExternalOutput")
        dbg_pt = nc.dram_tensor("dbg_pt", [IT, NT // 2, P, 2, 512], F8, kind="ExternalOutput")
        dbg_pr = nc.dram_tensor("dbg_pr", [IT, NT // 2, P, 2, 512], F8, kind="ExternalOutput")

    Exp = mybir.ActivationFunctionType.Exp
    Identity = mybir.ActivationFunctionType.Identity
    Copy = mybir.ActivationFunctionType.Copy
    Add = mybir.AluOpType.add
    Sub = mybir.AluOpType.subtract

    with TileContext(nc) as tc:
        with ExitStack() as top:
            const = top.enter_context(tc.tile_pool(name="const", bufs=1))
            kqv = top.enter_context(tc.tile_pool(name="kqv", bufs=1))
            ps_s = top.enter_context(tc.tile_pool(name="pss", bufs=2, space="PSUM"))
            ps_rs = top.enter_context(tc.tile_pool(name="psrs", bufs=1, space="PSUM"))

            ones8 = const.tile([P, 2, 1], F8, tag="ones")
            nc.vector.memset(ones8[:], WS)  # rowsum in the same 16x scale as V
            bq_sb = const.tile([P, 8], F32, tag="bq")
            bv_sb = const.tile([P, H], F32, tag="bv")

            kp = [kqv.tile([P, 2, N], F8, tag=f"kp{i}", name=f"kp{i}") for i in range(HP)]
            qp = [kqv.tile([P, 2, N], F8, tag=f"qp{i}", name=f"qp{i}") for i in range(HP)]
            qrp = (
                [kqv.tile([P, 2, N], F8, tag=f"qrp{i}", name=f"qrp{i}") for i in range(HP)]
                if qres
                else None
            )
            vp = [kqv.tile([P, 2, H], F8, tag=f"vp{u}", name=f"vp{u}") for u in range(NT // 2)]
            vrp = [kqv.tile([P, 2, H], F8, tag=f"vrp{u}", name=f"vrp{u}") for u in range(NT // 2)]

            # ---------------- Phase 1: projections (Q, K, then V) ----------------
            with ExitStack() as p1:
                xpool = p1.enter_context(tc.tile_pool(name="xp", bufs=1))
                wpool = p1.enter_context(tc.tile_pool(name="wp", bufs=1))
                ps1 = p1.enter_context(tc.tile_pool(name="ps1", bufs=5, space="PSUM"))

                xp = [xpool.tile([P, 2, N], F8, tag=f"x{d}", name=f"x{d}") for d in range(DP)]
                xrp = [xpool.tile([P, 2, N], F8, tag=f"xr{d}", name=f"xr{d}") for d in range(DP)]
                wq = [wpool.tile([P, 2, H], F8, tag=f"wq{d}", name=f"wq{d}") for d in range(DP)]
                wk = [wpool.tile([P, 2, H], F8, tag=f"wk{d}", name=f"wk{d}") for d in range(DP)]
                wv = [wpool.tile([P, 2, H], F8, tag=f"wv{d}", name=f"wv{d}") for d in range(DP)]
                wvr = [wpool.tile([P, 2, H], F8, tag=f"wvr{d}", name=f"wvr{d}") for d in range(DP)]

                def load_x_chunk(tiles, dram, nch):
                    cs = slice(nch * 512, (nch + 1) * 512)
                    for d in range(DP):
                        nc.sync.dma_start(tiles[d][:, :, cs], dram.ap()[d, :, :, cs])

                # DMA waves ordered to unblock the Q projection's first psum
                # group (needs wq + x/xr column chunk 0) as early as possible.
                # First wq tile split by partition quarters so several DMA
                # queues deliver it concurrently.
                for quarter in range(4):
                    qs = slice(quarter * 32, (quarter + 1) * 32)
                    nc.sync.dma_start(wq[0][qs, :, :], wq8d.ap()[0, qs, :, :])
                for d in range(1, DP):
                    nc.sync.dma_start(wq[d][:], wq8d.ap()[d])
                load_x_chunk(xp, x8d, 0)
                if qk_chains >= 2:
                    load_x_chunk(xrp, xr8d, 0)
                nc.sync.dma_start(bq_sb[:], bqS.ap()[:, :])
                for nch in range(1, 4):
                    load_x_chunk(xp, x8d, nch)
                    if qk_chains >= 2:
                        load_x_chunk(xrp, xr8d, nch)
                for d in range(DP):
                    nc.sync.dma_start(wk[d][:], wk8d.ap()[d])
                if qk_chains < 2:
                    for nch in range(4):
                        load_x_chunk(xrp, xr8d, nch)
                for d in range(DP):
                    nc.sync.dma_start(wv[d][:], wv8d.ap()[d])
                for d in range(DP):
                    nc.sync.dma_start(wvr[d][:], wvr8d.ap()[d])
                nc.sync.dma_start(bv_sb[:], bvB.ap()[:, :])

                def proj_qk(wtiles, evict):
                    srcs = [xp, xrp][:qk_chains]
                    total = DP * len(srcs)
                    for nch in range(4):
                        cs = slice(nch * 512, (nch + 1) * 512)
                        for hb in range(8):
                            ps = ps1.tile([P, 512], F32, tag="ps")
                            mm = 0
                            for src in srcs:
                                for d in range(DP):
                                    nc.tensor.matmul(
                                        ps[:],
                                        wtiles[d][:, :, hb * P:(hb + 1) * P],
                                        src[d][:, :, cs],
                                        start=(mm == 0),
                                        stop=(mm == total - 1),
                                        perf_mode=DR,
                                    )
                                    mm += 1
                            evict(ps, hb, nch)

                def evict_q(ps, hb, nch):
                    cs = slice(nch * 512, (nch + 1) * 512)
                    dst = qp[hb >> 1][:, hb & 1, cs]
                    nc.scalar.activation(dst, ps[:], Identity, bias=bq_sb[:, hb:hb + 1])
                    if qres:
                        nc.vector.scalar_tensor_tensor(
                            qrp[hb >> 1][:, hb & 1, cs],
                            ps[:],
                            bq_sb[:, hb:hb + 1],
                            dst,
                            op0=Add,
                            op1=Sub,
                        )

                def evict_k(ps, hb, nch):
                    cs = slice(nch * 512, (nch + 1) * 512)
                    nc.scalar.activation(kp[hb >> 1][:, hb & 1, cs], ps[:], Identity)

                proj_qk(wq, evict_q)
                proj_qk(wk, evict_k)

                # --- V = x @ Wv, 3-chain, kept resident as v8 + vr8 ---
                for nb in range(NT):
                    ns = slice(nb * P, (nb + 1) * P)
                    for hch in range(2):
                        hs = slice(hch * 512, (hch + 1) * 512)
                        ps = ps1.tile([P, 512], F32, tag="ps")
                        mm = 0
                        for (src_l, src_r) in ((xp, wv), (xrp, wv), (xp, wvr)):
                            for d in range(DP):
                                nc.tensor.matmul(
                                    ps[:],
                                    src_l[d][:, :, ns],
                                    src_r[d][:, :, hs],
                                    start=(mm == 0),
                                    stop=(mm == 3 * DP - 1),
                                    perf_mode=DR,
                                )
                                mm += 1
                        v8dst = vp[nb >> 1][:, nb & 1, hs]
                        nc.scalar.activation(v8dst, ps[:], Identity)
                        nc.vector.tensor_sub(vrp[nb >> 1][:, nb & 1, hs], ps[:], v8dst)

            if debug_dump:
                for i in range(HP):
                    nc.sync.dma_start(dbg_q.ap()[i], qp[i][:])
                    nc.sync.dma_start(dbg_k.ap()[i], kp[i][:])
                    if qres:
                        nc.sync.dma_start(dbg_qr.ap()[i], qrp[i][:])
                for u in range(NT // 2):
                    nc.sync.dma_start(dbg_v.ap()[u], vp[u][:])
                    nc.sync.dma_start(dbg_vr.ap()[u], vrp[u][:])

            # ---------------- Phase 2: attention ----------------
            with ExitStack() as p2:
                pt_pool = p2.enter_context(tc.tile_pool(name="pt", bufs=1))
                sm = p2.enter_context(tc.tile_pool(name="sm", bufs=4))
                op_pool = p2.enter_context(tc.tile_pool(name="op", bufs=2))
                ps_av = p2.enter_context(tc.tile_pool(name="psav", bufs=4, space="PSUM"))

                ptp = [pt_pool.tile([P, 2, 512], F8, tag=f"pt{u}", name=f"pt{u}") for u in range(NT // 2)]
                prp = [pt_pool.tile([P, 2, 512], F8, tag=f"pr{u}", name=f"pr{u}") for u in range(NT // 2)]

                for t in range(IT):
                    i0 = 512 * t
                    jmax = 4 * t + 3

                    # scores^T [key j, query i] + exp + causal mask, split
                    # into p8 + pr8 via a bf16 staging tile.
                    for j in range(jmax + 1):
                        c = max(0, j * P - i0)
                        w = 512 - c
                        u, m = j >> 1, j & 1
                        if c > 0:
                            # causally-dead columns: AV pair-matmuls read them
                            nc.gpsimd.memset(ptp[u][:, m, 0:c], 0.0)
                            nc.gpsimd.memset(prp[u][:, m, 0:c], 0.0)
                        ps = ps_s.tile([P, 512], F32, tag="ps")
                        for hp_ in range(HP):
                            nc.tensor.matmul(
                                ps[:, 0:w],
                                kp[hp_][:, :, j * P:(j + 1) * P],
                                qp[hp_][:, :, i0 + c:i0 + 512],
                                start=(hp_ == 0),
                                stop=(hp_ == HP - 1 and not qres),
                                perf_mode=DR,
                            )
                        if qres:
                            for hp_ in range(HP):
                                nc.tensor.matmul(
                                    ps[:, 0:w],
                                    kp[hp_][:, :, j * P:(j + 1) * P],
                                    qrp[hp_][:, :, i0 + c:i0 + 512],
                                    start=False,
                                    stop=(hp_ == HP - 1),
                                    perf_mode=DR,
                                )
                        pb = sm.tile([P, 512], BF16, tag="pbf", name="pbf")
                        nc.scalar.activation(pb[:, 0:w], ps[:, 0:w], Exp, scale=float(EXP_SCALE))
                        if c > 0 or j * P == i0:
                            # keep exp where key j*P+p <= query i0+c+f', else 0
                            nc.gpsimd.affine_select(
                                out=pb[:, 0:w],
                                in_=pb[:, 0:w],
                                compare_op=mybir.AluOpType.is_ge,
                                fill=0.0,
                                base=0,
                                channel_multiplier=-1,
                                pattern=[[1, w]],
                            )
                        nc.vector.tensor_copy(ptp[u][:, m, c:512], pb[:, 0:w])
                        nc.vector.tensor_sub(prp[u][:, m, c:512], pb[:, 0:w], ptp[u][:, m, c:512])

                    if debug_dump:
                        for u in range((jmax + 1) // 2):
                            nc.sync.dma_start(dbg_pt.ap()[t, u], ptp[u][:])
                            nc.sync.dma_start(dbg_pr.ap()[t, u], prp[u][:])

                    # attn @ V (3 fp8 chains), row-sums, normalize + bias on
                    # eviction
                    for s_ in range(4):
                        g = 4 * t + s_
                        qs = slice(s_ * P, (s_ + 1) * P)
                        umax = g >> 1
                        pav = [ps_av.tile([P, 512], F32, tag="pav", name="pav") for _ in range(2)]
                        prs = ps_rs.tile([P, 1], F32, tag="prs")
                        for u in range(umax + 1):
                            first = u == 0
                            last = u == umax
                            for hch in range(2):
                                hs = slice(hch * 512, (hch + 1) * 512)
                                nc.tensor.matmul(
                                    pav[hch][:], ptp[u][:, :, qs], vp[u][:, :, hs],
                                    start=first, stop=False, perf_mode=DR,
                                )
                                nc.tensor.matmul(
                                    pav[hch][:], ptp[u][:, :, qs], vrp[u][:, :, hs],
                                    start=False, stop=False, perf_mode=DR,
                                )
                                nc.tensor.matmul(
                                    pav[hch][:], prp[u][:, :, qs], vp[u][:, :, hs],
                                    start=False, stop=last, perf_mode=DR,
                                )
                            nc.tensor.matmul(
                                prs[:], ptp[u][:, :, qs], ones8[:],
                                start=first, stop=False, perf_mode=DR,
                            )
                            nc.tensor.matmul(
                                prs[:], prp[u][:, :, qs], ones8[:],
                                start=False, stop=last, perf_mode=DR,
                            )
                        recip = sm.tile([P, 1], F32, tag="recip")
                        nc.vector.reciprocal(recip[:], prs[:])
                        ot = op_pool.tile([P, H], F32, tag="ot")
                        for hch in range(2):
                            hs = slice(hch * 512, (hch + 1) * 512)
                            nc.scalar.activation(ot[:, hs], pav[hch][:], Copy, scale=recip[:])
                            nc.vector.tensor_add(ot[:, hs], ot[:, hs], bv_sb[:, hs])
                        nc.sync.dma_start(out.ap()[i0 + s_ * P:i0 + (s_ + 1) * P, :], ot[:])

    nc.compile()
    return nc


def _get_program():
    key = (QK_CHAINS, QRES)
    if key not in _CACHE:
        _CACHE[key] = build_program(*key)
    return _CACHE[key]


def _pair_layout(mat_t):
    """[D, N] (already fp8) -> [DP, P, 2, N] pair layout, contiguous."""
    d, n_ = mat_t.shape
    return np.ascontiguousarray(mat_t.reshape(DP, 2, P, n_).transpose(0, 2, 1, 3))


def prep_inputs(x, Wq, bq, Wk, bk, Wv, bv):
    x = np.asarray(x, dtype=np.float32)
    Wq = np.asarray(Wq, dtype=np.float32)
    Wk = np.asarray(Wk, dtype=np.float32)
    Wv = np.asarray(Wv, dtype=np.float32)
    bq = np.asarray(bq, dtype=np.float32)
    bv = np.asarray(bv, dtype=np.float32)

    def wprep(Wmat):
        Ws = Wmat * np.float32(WS)
        W8 = Ws.astype(F8NP)
        Wr8 = (Ws - W8.astype(np.float32)).astype(F8NP)
        return W8, Wr8

    Wq8, _ = wprep(Wq)
    Wk8, _ = wprep(Wk)
    Wv8, Wvr8 = wprep(Wv)
    # weight tiles contract over D: rows of W (no transpose) are the pair dim
    wq_l = _pair_layout(Wq8)
    wk_l = _pair_layout(Wk8)
    wv_l = _pair_layout(Wv8)
    wvr_l = _pair_layout(Wvr8)

    bqS_h = np.ascontiguousarray((bq * np.float32(WS)).reshape(8, P).T)
    bvB_h = np.ascontiguousarray(np.broadcast_to(bv, (P, H))).astype(np.float32)

    in_maps = []
    for b in range(B):
        xb = x[b]
        x8 = xb.astype(F8NP)
        xr8 = (xb - x8.astype(np.float32)).astype(F8NP)
        in_maps.append(
            {
                "x8d": _pair_layout(np.ascontiguousarray(x8.T)),
                "xr8d": _pair_layout(np.ascontiguousarray(xr8.T)),
                "wq8d": wq_l,
                "wk8d": wk_l,
                "wv8d": wv_l,
                "wvr8d": wvr_l,
                "bqS": bqS_h,
                "bvB": bvB_h,
            }
        )
    return in_maps


def kernel(x, Wq, bq, Wk, bk, Wv, bv):
    global LAST_RESULT
    nc = _get_program()
    in_maps = prep_inputs(x, Wq, bq, Wk, bk, Wv, bv)
    res = run_bass_kernel_spmd(nc, in_maps, core_ids=list(range(B)))
    LAST_RESULT = res
    return np.stack([res.results[b]["out"] for b in range(B)], axis=0)


# revision 11
# speedup vs baseline: 1.6467x; 1.6467x over previous
"""Causal single-head attention (B=8, N=2048, D=H=1024, fp32) on 8 TRN2 cores.

Data-parallel: one batch element per NeuronCore. Mixed fp8/bf16 design tuned
to the measured TRN2 PE behavior (1 moving column per cycle regardless of
dtype; fp8 DoubleRow contracts 256/instruction = 2x bf16 MACs, with weight
loads fully overlapped):

  q^T, k^T = fp8(e4m3) DoubleRow projections from x8/W8 (weights pre-scaled
             by 16 out of e4m3's subnormal range; K bias dropped — it is
             softmax-invariant)
  scores   = fp8 DoubleRow q8.k8 (single chain)
  p        = exp(scores * 2^-13) evicted straight to bf16
  V        = plain bf16 projection (elementwise-accurate; V errors hit the
             output directly through attention-concentrated rows)
  out      = bf16 p @ V / rowsum + bv   (bias applied post-normalization —
             it commutes with the attention average)

Everything stays resident in SBUF (no DRAM spills). Rowsums ride the PE as
1-column matmuls against a ones vector; the softmax division is folded into
the output eviction as a per-partition scale.

ATTN_QK_CHAINS=2 adds an x-residual chain to the Q/K projections (more
accuracy, ~55us slower).
"""

import os
import sys
from contextlib import ExitStack

import numpy as np
import ml_dtypes

# The concourse/bass toolchain comes from the container's python path; fall
# back to the /opt copy when running outside the preconfigured interpreter.
try:
    import concourse.bacc as bacc
except ImportError:  # pragma: no cover
    sys.path.insert(0, "/opt/trn_rl_repo")
    import concourse.bacc as bacc

import concourse.mybir as mybir
from concourse.tile import TileContext
from concourse.bass_utils import run_bass_kernel_spmd

# bass_utils imports antenv.axon_hooks when BASS_TRACE is set; provide a stub
# so tracing degrades gracefully instead of crashing if the module is absent.
try:
    import antenv.axon_hooks  # noqa: F401
except ImportError:  # pragma: no cover
    import types

    _m = types.ModuleType("antenv.axon_hooks")
    _m._hook = None
    _m.set_axon_ntff_profile_hook = lambda h: setattr(_m, "_hook", h)
    _m.get_axon_ntff_profile_hook = lambda: _m._hook
    sys.modules["antenv.axon_hooks"] = _m

# The boot-time NTFF hook install degrades silently when the image's antenv
# lacks axon_hooks; re-attempt it against our stub so BASS_TRACE captures
# HW profiles. Harmless no-op when axon or the .so is absent.
try:  # pragma: no cover
    import antenv.axon_hooks as _ah

    if _ah.get_axon_ntff_profile_hook() is None:
        from trn_agent_boot.trn_boot import _ntff_profile_via_ctypes

        _hook = _ntff_profile_via_ctypes("/opt/axon/libaxon_pjrt.so")
        if _hook is not None:
            _ah.set_axon_ntff_profile_hook(_hook)
except Exception:
    pass

B, N, D, H = 8, 2048, 1024, 1024
P = 128
DP = D // (2 * P)    # 4 fp8 contraction pair-tiles (256 deep each)
DT = D // P          # 8 bf16 contraction tiles
HP = H // (2 * P)    # 4 h pair-tiles for the score contraction
NT = N // P          # 16 sequence tiles of 128
IT = N // 512        # 4 query tiles of 512
WS = 16.0            # weight pre-scale: keeps W out of e4m3 subnormals
EXP_SCALE = 1.0 / (np.sqrt(float(H)) * WS * WS)  # 2^-13

F32 = mybir.dt.float32
F8 = mybir.dt.float8e4
BF16 = mybir.dt.bfloat16
F8NP = ml_dtypes.float8_e4m3
BFNP = ml_dtypes.bfloat16
DR = mybir.MatmulPerfMode.DoubleRow

QK_CHAINS = int(os.environ.get("ATTN_QK_CHAINS", "1"))  # 1 or 2

LAST_RESULT = None  # BassKernelResults of the most recent kernel() call
_CACHE = {}


def build_program(qk_chains: int = QK_CHAINS):
    nc = bacc.Bacc("TRN2", target_bir_lowering=False, debug=False)

    x8d = nc.dram_tensor("x8d", [DP, P, 2, N], F8, kind="ExternalInput")
    if qk_chains >= 2:
        xr8d = nc.dram_tensor("xr8d", [DP, P, 2, N], F8, kind="ExternalInput")
    xbd = nc.dram_tensor("xbd", [DT, P, N], BF16, kind="ExternalInput")
    wq8d = nc.dram_tensor("wq8d", [DP, P, 2, H], F8, kind="ExternalInput")
    wk8d = nc.dram_tensor("wk8d", [DP, P, 2, H], F8, kind="ExternalInput")
    wvbd = nc.dram_tensor("wvbd", [DT, P, H], BF16, kind="ExternalInput")
    bqS = nc.dram_tensor("bqS", [P, 8], F32, kind="ExternalInput")
    bvB = nc.dram_tensor("bvB", [P, H], F32, kind="ExternalInput")
    out = nc.dram_tensor("out", [N, H], F32, kind="ExternalOutput")

    Exp = mybir.ActivationFunctionType.Exp
    Identity = mybir.ActivationFunctionType.Identity
    Copy = mybir.ActivationFunctionType.Copy

    with TileContext(nc) as tc:
        with ExitStack() as top:
            const = top.enter_context(tc.tile_pool(name="const", bufs=1))
            kqv = top.enter_context(tc.tile_pool(name="kqv", bufs=1))
            ps_s = top.enter_context(tc.tile_pool(name="pss", bufs=2, space="PSUM"))
            ps_rs = top.enter_context(tc.tile_pool(name="psrs", bufs=1, space="PSUM"))

            ones_bf = const.tile([P, 1], BF16, tag="ones")
            nc.vector.memset(ones_bf[:], 1.0)
            bq_sb = const.tile([P, 8], F32, tag="bq")
            bv_sb = const.tile([P, H], F32, tag="bv")

            kp = [kqv.tile([P, 2, N], F8, tag=f"kp{i}", name=f"kp{i}") for i in range(HP)]
            qp = [kqv.tile([P, 2, N], F8, tag=f"qp{i}", name=f"qp{i}") for i in range(HP)]
            vt = [kqv.tile([P, H], BF16, tag=f"vt{j}", name=f"vt{j}") for j in range(NT)]

            # ---------------- Phase 1: projections (Q, K, then V) ----------------
            with ExitStack() as p1:
                xpool = p1.enter_context(tc.tile_pool(name="xp", bufs=1))
                wpool = p1.enter_context(tc.tile_pool(name="wp", bufs=1))
                ps1 = p1.enter_context(tc.tile_pool(name="ps1", bufs=5, space="PSUM"))

                xp = [xpool.tile([P, 2, N], F8, tag=f"x{d}", name=f"x{d}") for d in range(DP)]
                if qk_chains >= 2:
                    xrp = [xpool.tile([P, 2, N], F8, tag=f"xr{d}", name=f"xr{d}") for d in range(DP)]
                xb = [xpool.tile([P, N], BF16, tag=f"xb{d}", name=f"xb{d}") for d in range(DT)]
                wq = [wpool.tile([P, 2, H], F8, tag=f"wq{d}", name=f"wq{d}") for d in range(DP)]
                wk = [wpool.tile([P, 2, H], F8, tag=f"wk{d}", name=f"wk{d}") for d in range(DP)]
                wvb = [wpool.tile([P, H], BF16, tag=f"wvb{d}", name=f"wvb{d}") for d in range(DT)]

                def load_x_chunk(tiles, dram, nch):
                    cs = slice(nch * 512, (nch + 1) * 512)
                    for d in range(DP):
                        nc.sync.dma_start(tiles[d][:, :, cs], dram.ap()[d, :, :, cs])

                # DMA waves ordered to unblock the Q projection's first psum
                # group (needs wq + x8 column chunk 0) as early as possible;
                # its d=0 pair (gating the first matmul) goes first, split by
                # partition quarters across DMA queues.
                for quarter in range(4):
                    qsl = slice(quarter * 32, (quarter + 1) * 32)
                    nc.sync.dma_start(wq[0][qsl, :, :], wq8d.ap()[0, qsl, :, :])
                    nc.sync.dma_start(xp[0][qsl, :, 0:512], x8d.ap()[0, qsl, :, 0:512])
                for d in range(1, DP):
                    nc.sync.dma_start(wq[d][:], wq8d.ap()[d])
                    nc.sync.dma_start(xp[d][:, :, 0:512], x8d.ap()[d, :, :, 0:512])
                if qk_chains >= 2:
                    load_x_chunk(xrp, xr8d, 0)
                nc.sync.dma_start(bq_sb[:], bqS.ap()[:, :])
                for nch in range(1, 4):
                    load_x_chunk(xp, x8d, nch)
                    if qk_chains >= 2:
                        load_x_chunk(xrp, xr8d, nch)
                for d in range(DP):
                    nc.sync.dma_start(wk[d][:], wk8d.ap()[d])
                for d in range(DT):
                    nc.sync.dma_start(xb[d][:], xbd.ap()[d])
                for d in range(DT):
                    nc.sync.dma_start(wvb[d][:], wvbd.ap()[d])
                nc.sync.dma_start(bv_sb[:], bvB.ap()[:, :])

                def proj_qk(wtiles, evict):
                    srcs = [xp, xrp][:qk_chains] if qk_chains >= 2 else [xp]
                    total = DP * len(srcs)
                    for nch in range(4):
                        cs = slice(nch * 512, (nch + 1) * 512)
                        for hb in range(8):
                            ps = ps1.tile([P, 512], F32, tag="ps")
                            mm = 0
                            for src in srcs:
                                for d in range(DP):
                                    nc.tensor.matmul(
                                        ps[:],
                                        wtiles[d][:, :, hb * P:(hb + 1) * P],
                                        src[d][:, :, cs],
                                        start=(mm == 0),
                                        stop=(mm == total - 1),
                                        perf_mode=DR,
                                    )
                                    mm += 1
                            evict(ps, hb, nch)

                def evict_q(ps, hb, nch):
                    cs = slice(nch * 512, (nch + 1) * 512)
                    nc.scalar.activation(
                        qp[hb >> 1][:, hb & 1, cs], ps[:], Identity, bias=bq_sb[:, hb:hb + 1]
                    )

                def evict_k(ps, hb, nch):
                    cs = slice(nch * 512, (nch + 1) * 512)
                    nc.vector.tensor_copy(kp[hb >> 1][:, hb & 1, cs], ps[:])

                proj_qk(wq, evict_q)
                proj_qk(wk, evict_k)

                # --- V = x @ Wv in bf16, kept resident (no bias — added at
                # the end, where it commutes with the attention average) ---
                for nb in range(NT):
                    ns = slice(nb * P, (nb + 1) * P)
                    for hch in range(2):
                        hs = slice(hch * 512, (hch + 1) * 512)
                        ps = ps1.tile([P, 512], F32, tag="ps")
                        for d in range(DT):
                            nc.tensor.matmul(
                                ps[:],
                                xb[d][:, ns],
                                wvb[d][:, hs],
                                start=(d == 0),
                                stop=(d == DT - 1),
                            )
                        nc.scalar.activation(vt[nb][:, hs], ps[:], Identity)

            # ---------------- Phase 2: attention ----------------
            with ExitStack() as p2:
                pt_pool = p2.enter_context(tc.tile_pool(name="pt", bufs=1))
                sm = p2.enter_context(tc.tile_pool(name="sm", bufs=4))
                op_pool = p2.enter_context(tc.tile_pool(name="op", bufs=2))
                ps_av = p2.enter_context(tc.tile_pool(name="psav", bufs=4, space="PSUM"))

                pt = [pt_pool.tile([P, 512], BF16, tag=f"pt{j}", name=f"pt{j}") for j in range(NT)]

                for t in range(IT):
                    i0 = 512 * t
                    jmax = 4 * t + 3

                    # scores^T [key j, query i] -> exp -> bf16 p, causal mask
                    # on the diagonal tiles. Columns below the diagonal cut c
                    # are never read by this t's AV matmuls.
                    for j in range(jmax + 1):
                        c = max(0, j * P - i0)
                        w = 512 - c
                        ps = ps_s.tile([P, 512], F32, tag="ps")
                        for hp_ in range(HP):
                            nc.tensor.matmul(
                                ps[:, 0:w],
                                kp[hp_][:, :, j * P:(j + 1) * P],
                                qp[hp_][:, :, i0 + c:i0 + 512],
                                start=(hp_ == 0),
                                stop=(hp_ == HP - 1),
                                perf_mode=DR,
                            )
                        nc.scalar.activation(pt[j][:, c:512], ps[:, 0:w], Exp, scale=float(EXP_SCALE))
                        if c > 0 or j * P == i0:
                            # keep exp where key j*P+p <= query i0+c+f', else 0
                            nc.gpsimd.affine_select(
                                out=pt[j][:, c:512],
                                in_=pt[j][:, c:512],
                                compare_op=mybir.AluOpType.is_ge,
                                fill=0.0,
                                base=0,
                                channel_multiplier=-1,
                                pattern=[[1, w]],
                            )

                    # attn @ V, row-sums, normalize + bias on eviction
                    for s_ in range(4):
                        g = 4 * t + s_
                        qs = slice(s_ * P, (s_ + 1) * P)
                        pav = [ps_av.tile([P, 512], F32, tag="pav", name="pav") for _ in range(2)]
                        prs = ps_rs.tile([P, 1], F32, tag="prs")
                        for j in range(g + 1):
                            lhsT = pt[j][:, qs]
                            for hch in range(2):
                                nc.tensor.matmul(
                                    pav[hch][:],
                                    lhsT,
                                    vt[j][:, hch * 512:(hch + 1) * 512],
                                    start=(j == 0),
                                    stop=(j == g),
                                )
                            nc.tensor.matmul(
                                prs[:], lhsT, ones_bf[:], start=(j == 0), stop=(j == g)
                            )
                        recip = sm.tile([P, 1], F32, tag="recip")
                        nc.vector.reciprocal(recip[:], prs[:])
                        ot = op_pool.tile([P, H], F32, tag="ot")
                        for hch in range(2):
                            hs = slice(hch * 512, (hch + 1) * 512)
                            nc.scalar.activation(ot[:, hs], pav[hch][:], Copy, scale=recip[:])
                            nc.vector.tensor_add(ot[:, hs], ot[:, hs], bv_sb[:, hs])
                            nc.sync.dma_start(
                                out.ap()[i0 + s_ * P:i0 + (s_ + 1) * P, hs], ot[:, hs]
                            )

    nc.compile()
    return nc


def _get_program():
    key = QK_CHAINS
    if key not in _CACHE:
        _CACHE[key] = build_program(key)
    return _CACHE[key]


def _pair_layout(mat):
    """[D, M] (already fp8) -> [DP, P, 2, M] pair layout, contiguous."""
    d, m_ = mat.shape
    return np.ascontiguousarray(mat.reshape(DP, 2, P, m_).transpose(0, 2, 1, 3))


def prep_inputs(x, Wq, bq, Wk, bk, Wv, bv, qk_chains: int = None):
    if qk_chains is None:
        qk_chains = QK_CHAINS
    x = np.asarray(x, dtype=np.float32)
    Wq = np.asarray(Wq, dtype=np.float32)
    Wk = np.asarray(Wk, dtype=np.float32)
    Wv = np.asarray(Wv, dtype=np.float32)
    bq = np.asarray(bq, dtype=np.float32)
    bv = np.asarray(bv, dtype=np.float32)

    # weight tiles contract over D: rows of W (no transpose) are the pair dim
    wq_l = _pair_layout((Wq * np.float32(WS)).astype(F8NP))
    wk_l = _pair_layout((Wk * np.float32(WS)).astype(F8NP))
    wvb_l = np.ascontiguousarray(Wv.astype(BFNP).reshape(DT, P, H))

    bqS_h = np.ascontiguousarray((bq * np.float32(WS)).reshape(8, P).T)
    bvB_h = np.ascontiguousarray(np.broadcast_to(bv, (P, H))).astype(np.float32)

    in_maps = []
    for b in range(B):
        xb_ = x[b]
        x8 = xb_.astype(F8NP)
        m = {
            "x8d": _pair_layout(np.ascontiguousarray(x8.T)),
            "xbd": np.ascontiguousarray(xb_.T.astype(BFNP).reshape(DT, P, N)),
            "wq8d": wq_l,
            "wk8d": wk_l,
            "wvbd": wvb_l,
            "bqS": bqS_h,
            "bvB": bvB_h,
        }
        if qk_chains >= 2:
            xr8 = (xb_ - x8.astype(np.float32)).astype(F8NP)
            m["xr8d"] = _pair_layout(np.ascontiguousarray(xr8.T))
        in_maps.append(m)
    return in_maps


def kernel(x, Wq, bq, Wk, bk, Wv, bv):
    global LAST_RESULT
    nc = _get_program()
    in_maps = prep_inputs(x, Wq, bq, Wk, bk, Wv, bv)
    res = run_bass_kernel_spmd(nc, in_maps, core_ids=list(range(B)))
    LAST_RESULT = res
    return np.stack([res.results[b]["out"] for b in range(B)], axis=0)
